# revision 17
# baseline (speedup 1.0000x reference)
"""Dcls1d (dilated conv1d with learnable spacings) on 8 Trainium2 NeuronCores.

Problem: x (8, 256, 2048) f32; weight (256, 256, 16); P (1, 256, 256, 16);
bias (256,). A dense conv kernel (O=256, I=256, DKS=33) is built from
weight/P by linear interpolation at positions P, then conv1d(x, kern,
pad=16) + bias -> out (8, 256, 2048).

Strategy:
 - Host: fold (weight, P) -> dense per-tap matmul weights, keeping only the
   taps that are actually nonzero (P = clip(0.5*randn, +-16), so positions
   cluster around the center tap 16: typically only ~7 of 33 taps carry any
   weight; the rest are exactly zero and contribute nothing to the conv).
 - Device: data-parallel over batch, one batch element per NeuronCore. The
   conv is a sum over taps d of kern[d].T @ x shifted by d, accumulated in
   PSUM: per core 2x4 output tiles of (128, 512), each accumulating
   2*T matmuls (two 128-deep input-channel chunks x T taps), + bias via
   ScalarE Identity-activation on the PSUM->SBUF move.
"""

import os
import numpy as np

try:
    import concourse  # noqa: F401
except ImportError:  # pragma: no cover - container fallback
    import sys

    sys.path.insert(0, "/opt/trn_rl_repo")

import concourse.bacc as bacc
import concourse.mybir as mybir
import concourse.tile as tile
import concourse.bass_utils as bass_utils

DKS = 33
PAD = 16
N, IC, LEN = 8, 256, 2048
OC = 256
KC = 16
N_CORES = 8

TRACE = False  # test harness sets kernel_mod.TRACE = True to profile
LAST_EXEC_NS = None
LAST_TRACE_PATH = None

_BUILD_CACHE = {}


def _host_fold_kernel(weight, P):
    """Reproduce reference construct_kernel for the active taps only.

    Returns (taps_lo, ktaps) where ktaps[t, i, o] is the lhsT-layout matmul
    weight for tap d = taps_lo + t. All arithmetic mirrors the reference
    (fp32): kern[o,i,d] = sum_kc w[o,i,kc] * (W1 + frac*(W2-W1)).
    """
    w = np.asarray(weight, dtype=np.float32)
    Pf32 = np.asarray(P, dtype=np.float32)
    Pp = Pf32 + np.float32(DKS // 2)
    Pf = np.floor(Pp)
    frac = (Pp - Pf)[0, 0]  # (IC, KC) - out-channel 0's fractional part
    P1 = Pf[0]  # (OC, IC, KC)

    dmin = max(0, int(P1.min()))
    dmax = min(DKS - 1, int(P1.max()) + 1)
    dd = np.arange(dmin, dmax + 1, dtype=np.float32)
    W1 = dd[:, None, None, None] == P1[None]
    W2 = dd[:, None, None, None] == (P1 + 1)[None]
    K = W1.astype(np.float32) + frac[None, None] * (
        W2.astype(np.float32) - W1.astype(np.float32)
    )
    kern = (w[None] * K).sum(-1)  # (T, OC, IC)
    ktaps = np.ascontiguousarray(kern.transpose(0, 2, 1))  # (T, IC, OC)
    return dmin, ktaps


def _build(T, dmin, dmax):
    f32 = mybir.dt.float32
    f32r = mybir.dt.float32r

    W = LEN + dmax - dmin  # host-padded x width; tap d tc-chunk reads
    # columns [(d - dmin) + 512*tc, ...+512)

    n_tc = LEN // 512

    nc = bacc.Bacc("TRN2", target_bir_lowering=False, debug=False,
                   num_devices=N_CORES)
    x_d = nc.dram_tensor("x", (2, 128, W), f32r, kind="ExternalInput")
    kt_d = nc.dram_tensor("kt", (2, 128, T, OC), f32r, kind="ExternalInput")
    b_d = nc.dram_tensor("bias", (128, 2), f32, kind="ExternalInput")
    y_d = nc.dram_tensor("out", (2, 128, LEN), f32, kind="ExternalOutput")

    with tile.TileContext(nc) as tc:
        with (
            tc.tile_pool(name="const", bufs=1) as cpool,
            tc.tile_pool(name="ps", bufs=8, space="PSUM") as pspool,
            tc.tile_pool(name="outp", bufs=4) as opool,
        ):
            # DMA cost model here: ~600-700 ns serialized trigger per
            # dma_start on the issuing sequencer (HWDGE: sync + scalar), then
            # ~60-80 ns queue time per contiguous row. So: few transfers,
            # split across both HWDGE engines, ordered by PE need.
            kt_t = []
            for ic in range(2):
                t_ = cpool.tile([128, T, OC], f32r, tag=f"kt{ic}")
                kt_t.append(t_)
            xp = []
            for ic in range(2):
                t_ = cpool.tile([128, W], f32r, tag=f"xp{ic}")
                xp.append(t_)
            bias_t = cpool.tile([128, 2], f32, tag="bias")

            # PE warmup: the HAM clock gate holds the PE at 1.2 GHz until
            # it has been busy ~3.4us. Dummy back-to-back matmuls on scratch
            # data keep it busy during the DMA fill so real matmuls run at
            # 2.4 GHz from the start.
            warm = cpool.tile([128, 64], f32r, tag="warm")
            nc.vector.memset(warm[:].bitcast(f32), 0.0)
            wps = pspool.tile([64, 64], f32, tag="ps", name="warm_ps")
            for i in range(40):
                nc.tensor.matmul(wps[:], warm[:, 0:64], warm[:],
                                 start=True, stop=True)

            # DMA completion is ~serial per stream (each chunk's semaphore
            # fires ~2.5-3.5us after the previous one on its stream), so
            # order each stream by when the PE consumes the data. The matmul
            # stream is phase-split: first all 8 output groups' ic0 halves
            # (needing only kt0 + xp0), then the ic1 halves.
            dh = max(1, min(3, T - 1))  # first kt chunk: taps [0, dh)
            ch = 1024 + T - 1  # xp column split: tcn0+1 read [0, ch)
            nc.sync.dma_start(kt_t[0][:, 0:dh], kt_d.ap()[0][:, 0:dh])
            nc.scalar.dma_start(xp[0][:, 0:ch], x_d.ap()[0][:, 0:ch])
            nc.sync.dma_start(kt_t[0][:, dh:T], kt_d.ap()[0][:, dh:T])
            nc.scalar.dma_start(xp[0][:, ch:W], x_d.ap()[0][:, ch:W])
            nc.sync.dma_start(kt_t[1][:, 0:dh], kt_d.ap()[1][:, 0:dh])
            nc.scalar.dma_start(xp[1][:, 0:ch], x_d.ap()[1][:, 0:ch])
            nc.sync.dma_start(kt_t[1][:, dh:T], kt_d.ap()[1][:, dh:T])
            nc.scalar.dma_start(xp[1][:, ch:W], x_d.ap()[1][:, ch:W])
            nc.sync.dma_start(bias_t[:], b_d.ap())

            ps = {}
            for tcn in range(n_tc):
                for oc in range(2):
                    ps[tcn, oc] = pspool.tile([128, 512], f32, tag="ps",
                                              name=f"ps_{tcn}_{oc}")

            def mm(ic, tcn, oc, d):
                nc.tensor.matmul(
                    ps[tcn, oc][:],
                    kt_t[ic][:, d, oc * 128:(oc + 1) * 128],
                    xp[ic][:, d + tcn * 512:d + tcn * 512 + 512],
                    start=(ic == 0 and d == 0),
                    stop=(ic == 1 and d == T - 1),
                )

            for ic in range(2):
                # early-data sub-phase: first-half tcn chunks x first kt taps
                # run on just the first (small, early-arriving) DMA chunks
                for tcn in range(2):
                    for oc in range(2):
                        for d in range(dh):
                            mm(ic, tcn, oc, d)
                for tcn in range(2):
                    for oc in range(2):
                        for d in range(dh, T):
                            mm(ic, tcn, oc, d)
                for tcn in range(2, n_tc):
                    for oc in range(2):
                        for d in range(T):
                            mm(ic, tcn, oc, d)
                for tcn in range(n_tc):
                    for oc in range(2):
                        if ic == 0:
                            continue
                        ot = opool.tile([128, 512], f32, tag="out",
                                        name=f"ot_{tcn}_{oc}")
                        last = (tcn == n_tc - 1 and oc == 1)
                        if not last:
                            nc.vector.tensor_scalar(
                                ot[:], ps[tcn, oc][:], bias_t[:, oc:oc + 1],
                                None, mybir.AluOpType.add,
                            )
                            nc.sync.dma_start(
                                y_d.ap()[oc][:, tcn * 512:(tcn + 1) * 512],
                                ot[:],
                            )
                        else:
                            # split the final store across engines to trim
                            # the critical tail
                            nc.vector.tensor_scalar(
                                ot[:, 0:256], ps[tcn, oc][:, 0:256],
                                bias_t[:, oc:oc + 1], None,
                                mybir.AluOpType.add,
                            )
                            nc.scalar.activation(
                                ot[:, 256:512], ps[tcn, oc][:, 256:512],
                                mybir.ActivationFunctionType.Identity,
                                bias=bias_t[:, oc:oc + 1],
                            )
                            c0 = tcn * 512
                            nc.sync.dma_start(
                                y_d.ap()[oc][:, c0:c0 + 256], ot[:, 0:256]
                            )
                            nc.scalar.dma_start(
                                y_d.ap()[oc][:, c0 + 256:c0 + 512],
                                ot[:, 256:512]
                            )

    nc.compile()
    return nc


def kernel(x, weight, P, bias):
    global LAST_EXEC_NS, LAST_TRACE_PATH
    x = np.ascontiguousarray(np.asarray(x, dtype=np.float32))
    bias = np.asarray(bias, dtype=np.float32)

    dmin, ktaps = _host_fold_kernel(weight, P)
    T = ktaps.shape[0]
    dmax = dmin + T - 1

    key = (T, dmin, dmax)
    if key not in _BUILD_CACHE:
        _BUILD_CACHE[key] = _build(T, dmin, dmax)
    nc = _BUILD_CACHE[key]

    # kt layout (ic_chunk, i_within, tap, o) so per-ic rows are contiguous
    kt_in = np.ascontiguousarray(
        ktaps.reshape(T, 2, 128, OC).transpose(1, 2, 0, 3))
    b_in = np.ascontiguousarray(bias.reshape(2, 128).T)
    # pad x on the host so the device needs no memset: column c of xpad is
    # x index c + (dmin - PAD)
    W = LEN + dmax - dmin
    zl = max(0, PAD - dmin)
    xs = max(0, dmin - PAD)
    xn = min(LEN - xs, W - zl)
    xpad = np.zeros((N_CORES, 2, 128, W), dtype=np.float32)
    xpad[:, :, :, zl:zl + xn] = x.reshape(N_CORES, 2, 128, LEN)[:, :, :, xs:xs + xn]
    in_maps = [
        {
            "x": xpad[c],
            "kt": kt_in,
            "bias": b_in,
        }
        for c in range(N_CORES)
    ]

    kwargs = {}
    if TRACE:
        bass_utils.upload_artifacts = lambda tmpdir: tmpdir
        kwargs["trace"] = True
    res = bass_utils.run_bass_kernel_spmd(
        nc, in_maps, core_ids=list(range(N_CORES)), **kwargs
    )
    if TRACE:
        LAST_EXEC_NS = res.exec_time_ns
        if res.instructions_and_trace is not None:
            LAST_TRACE_PATH = res.instructions_and_trace[1]

    out = np.empty((N, OC, LEN), dtype=np.float32)
    for c in range(N_CORES):
        out[c] = res.results[c]["out"].reshape(OC, LEN)
    return out


# revision 18
# speedup vs baseline: 1.0280x; 1.0280x over previous
"""Dcls1d (dilated conv1d with learnable spacings) on 8 Trainium2 NeuronCores.

Problem: x (8, 256, 2048) f32; weight (256, 256, 16); P (1, 256, 256, 16);
bias (256,). A dense conv kernel (O=256, I=256, DKS=33) is built from
weight/P by linear interpolation at positions P, then conv1d(x, kern,
pad=16) + bias -> out (8, 256, 2048).

Strategy:
 - Host: fold (weight, P) -> dense per-tap matmul weights, keeping only the
   taps that are actually nonzero (P = clip(0.5*randn, +-16), so positions
   cluster around the center tap 16: typically only ~7 of 33 taps carry any
   weight; the rest are exactly zero and contribute nothing to the conv).
 - Device: data-parallel over batch, one batch element per NeuronCore. The
   conv is a sum over taps d of kern[d].T @ x shifted by d, accumulated in
   PSUM: per core 2x4 output tiles of (128, 512), each accumulating
   2*T matmuls (two 128-deep input-channel chunks x T taps), + bias via
   ScalarE Identity-activation on the PSUM->SBUF move.
"""

import os
import numpy as np

try:
    import concourse  # noqa: F401
except ImportError:  # pragma: no cover - container fallback
    import sys

    sys.path.insert(0, "/opt/trn_rl_repo")

import concourse.bacc as bacc
import concourse.mybir as mybir
import concourse.tile as tile
import concourse.bass_utils as bass_utils

DKS = 33
PAD = 16
N, IC, LEN = 8, 256, 2048
OC = 256
KC = 16
N_CORES = 8

TRACE = False  # test harness sets kernel_mod.TRACE = True to profile
DTYPE = "f32r"  # "f32r" (safe, ~1.5e-4 rel err) or "bf16" (faster, ~5e-3)
LAST_EXEC_NS = None
LAST_TRACE_PATH = None

_BUILD_CACHE = {}


def _host_fold_kernel(weight, P):
    """Reproduce reference construct_kernel for the active taps only.

    Returns (taps_lo, ktaps) where ktaps[t, i, o] is the lhsT-layout matmul
    weight for tap d = taps_lo + t. All arithmetic mirrors the reference
    (fp32): kern[o,i,d] = sum_kc w[o,i,kc] * (W1 + frac*(W2-W1)).
    """
    w = np.asarray(weight, dtype=np.float32)
    Pf32 = np.asarray(P, dtype=np.float32)
    Pp = Pf32 + np.float32(DKS // 2)
    Pf = np.floor(Pp)
    frac = (Pp - Pf)[0, 0]  # (IC, KC) - out-channel 0's fractional part
    P1 = Pf[0]  # (OC, IC, KC)

    dmin = max(0, int(P1.min()))
    dmax = min(DKS - 1, int(P1.max()) + 1)
    dd = np.arange(dmin, dmax + 1, dtype=np.float32)
    W1 = dd[:, None, None, None] == P1[None]
    W2 = dd[:, None, None, None] == (P1 + 1)[None]
    K = W1.astype(np.float32) + frac[None, None] * (
        W2.astype(np.float32) - W1.astype(np.float32)
    )
    kern = (w[None] * K).sum(-1)  # (T, OC, IC)
    ktaps = np.ascontiguousarray(kern.transpose(0, 2, 1))  # (T, IC, OC)
    return dmin, ktaps


def _build(T, dmin, dmax, dtype_name):
    f32 = mybir.dt.float32
    f32r = mybir.dt.float32r if dtype_name == "f32r" else mybir.dt.bfloat16

    W = LEN + dmax - dmin  # host-padded x width; tap d tc-chunk reads
    # columns [(d - dmin) + 512*tc, ...+512)

    n_tc = LEN // 512

    nc = bacc.Bacc("TRN2", target_bir_lowering=False, debug=False,
                   num_devices=N_CORES)
    x_d = nc.dram_tensor("x", (2, 128, W), f32r, kind="ExternalInput")
    kt_d = nc.dram_tensor("kt", (2, 128, T, OC), f32r, kind="ExternalInput")
    b_d = nc.dram_tensor("bias", (128, 2), f32, kind="ExternalInput")
    y_d = nc.dram_tensor("out", (2, 128, LEN), f32, kind="ExternalOutput")

    with tile.TileContext(nc) as tc:
        with (
            tc.tile_pool(name="const", bufs=1) as cpool,
            tc.tile_pool(name="ps", bufs=8, space="PSUM") as pspool,
            tc.tile_pool(name="outp", bufs=4) as opool,
        ):
            # DMA cost model here: ~600-700 ns serialized trigger per
            # dma_start on the issuing sequencer (HWDGE: sync + scalar), then
            # ~60-80 ns queue time per contiguous row. So: few transfers,
            # split across both HWDGE engines, ordered by PE need.
            kt_t = []
            for ic in range(2):
                t_ = cpool.tile([128, T, OC], f32r, tag=f"kt{ic}")
                kt_t.append(t_)
            xp = []
            for ic in range(2):
                t_ = cpool.tile([128, W], f32r, tag=f"xp{ic}")
                xp.append(t_)
            bias_t = cpool.tile([128, 2], f32, tag="bias")

            # PE warmup: the HAM clock gate holds the PE at 1.2 GHz until
            # it has been busy ~3.4us. Dummy back-to-back matmuls on scratch
            # data keep it busy during the DMA fill so real matmuls run at
            # 2.4 GHz from the start.
            warm = cpool.tile([128, 64], f32r, tag="warm")
            nc.vector.memset(warm[:].bitcast(f32 if dtype_name == 'f32r' else mybir.dt.bfloat16), 0.0)
            wps = pspool.tile([64, 64], f32, tag="ps", name="warm_ps")
            for i in range(40):
                nc.tensor.matmul(wps[:], warm[:, 0:64], warm[:],
                                 start=True, stop=True)

            # DMA completion is ~serial per stream (each chunk's semaphore
            # fires ~2.5-3.5us after the previous one on its stream), so
            # order each stream by when the PE consumes the data. The matmul
            # stream is phase-split: first all 8 output groups' ic0 halves
            # (needing only kt0 + xp0), then the ic1 halves.
            dh = max(1, min(3, T - 1))  # first kt chunk: taps [0, dh)
            ch = 1024 + T - 1  # xp column split: tcn0+1 read [0, ch)
            nc.sync.dma_start(kt_t[0][:, 0:dh], kt_d.ap()[0][:, 0:dh])
            nc.scalar.dma_start(xp[0][:, 0:ch], x_d.ap()[0][:, 0:ch])
            nc.sync.dma_start(kt_t[0][:, dh:T], kt_d.ap()[0][:, dh:T])
            nc.scalar.dma_start(xp[0][:, ch:W], x_d.ap()[0][:, ch:W])
            nc.sync.dma_start(kt_t[1][:, 0:dh], kt_d.ap()[1][:, 0:dh])
            nc.scalar.dma_start(xp[1][:, 0:ch], x_d.ap()[1][:, 0:ch])
            nc.sync.dma_start(kt_t[1][:, dh:T], kt_d.ap()[1][:, dh:T])
            nc.scalar.dma_start(xp[1][:, ch:W], x_d.ap()[1][:, ch:W])
            nc.sync.dma_start(bias_t[:], b_d.ap())

            ps = {}
            for tcn in range(n_tc):
                for oc in range(2):
                    ps[tcn, oc] = pspool.tile([128, 512], f32, tag="ps",
                                              name=f"ps_{tcn}_{oc}")

            def mm(ic, tcn, oc, d):
                nc.tensor.matmul(
                    ps[tcn, oc][:],
                    kt_t[ic][:, d, oc * 128:(oc + 1) * 128],
                    xp[ic][:, d + tcn * 512:d + tcn * 512 + 512],
                    start=(ic == 0 and d == 0),
                    stop=(ic == 1 and d == T - 1),
                )

            for ic in range(2):
                # early-data sub-phase: first-half tcn chunks x first kt taps
                # run on just the first (small, early-arriving) DMA chunks
                for tcn in range(2):
                    for oc in range(2):
                        for d in range(dh):
                            mm(ic, tcn, oc, d)
                for tcn in range(2):
                    for oc in range(2):
                        for d in range(dh, T):
                            mm(ic, tcn, oc, d)
                for tcn in range(2, n_tc):
                    for oc in range(2):
                        for d in range(T):
                            mm(ic, tcn, oc, d)
                for tcn in range(n_tc):
                    for oc in range(2):
                        if ic == 0:
                            continue
                        ot = opool.tile([128, 512], f32, tag="out",
                                        name=f"ot_{tcn}_{oc}")
                        last = (tcn == n_tc - 1 and oc == 1)
                        if not last:
                            nc.vector.tensor_scalar(
                                ot[:], ps[tcn, oc][:], bias_t[:, oc:oc + 1],
                                None, mybir.AluOpType.add,
                            )
                            nc.sync.dma_start(
                                y_d.ap()[oc][:, tcn * 512:(tcn + 1) * 512],
                                ot[:],
                            )
                        else:
                            # split the final store across engines to trim
                            # the critical tail
                            nc.vector.tensor_scalar(
                                ot[:, 0:256], ps[tcn, oc][:, 0:256],
                                bias_t[:, oc:oc + 1], None,
                                mybir.AluOpType.add,
                            )
                            nc.scalar.activation(
                                ot[:, 256:512], ps[tcn, oc][:, 256:512],
                                mybir.ActivationFunctionType.Identity,
                                bias=bias_t[:, oc:oc + 1],
                            )
                            c0 = tcn * 512
                            nc.sync.dma_start(
                                y_d.ap()[oc][:, c0:c0 + 256], ot[:, 0:256]
                            )
                            nc.scalar.dma_start(
                                y_d.ap()[oc][:, c0 + 256:c0 + 512],
                                ot[:, 256:512]
                            )

    nc.compile()
    return nc


def kernel(x, weight, P, bias):
    global LAST_EXEC_NS, LAST_TRACE_PATH
    x = np.ascontiguousarray(np.asarray(x, dtype=np.float32))
    bias = np.asarray(bias, dtype=np.float32)

    dmin, ktaps = _host_fold_kernel(weight, P)
    T = ktaps.shape[0]
    dmax = dmin + T - 1

    key = (T, dmin, dmax, DTYPE)
    if key not in _BUILD_CACHE:
        _BUILD_CACHE[key] = _build(T, dmin, dmax, DTYPE)
    nc = _BUILD_CACHE[key]

    # kt layout (ic_chunk, i_within, tap, o) so per-ic rows are contiguous
    kt_in = np.ascontiguousarray(
        ktaps.reshape(T, 2, 128, OC).transpose(1, 2, 0, 3))
    if DTYPE == "bf16":
        import ml_dtypes
        kt_in = kt_in.astype(ml_dtypes.bfloat16)
    b_in = np.ascontiguousarray(bias.reshape(2, 128).T)
    # pad x on the host so the device needs no memset: column c of xpad is
    # x index c + (dmin - PAD)
    W = LEN + dmax - dmin
    zl = max(0, PAD - dmin)
    xs = max(0, dmin - PAD)
    xn = min(LEN - xs, W - zl)
    xdt = np.float32
    if DTYPE == "bf16":
        import ml_dtypes
        xdt = ml_dtypes.bfloat16
    xpad = np.zeros((N_CORES, 2, 128, W), dtype=xdt)
    xpad[:, :, :, zl:zl + xn] = x.reshape(N_CORES, 2, 128, LEN)[:, :, :, xs:xs + xn].astype(xdt)
    in_maps = [
        {
            "x": xpad[c],
            "kt": kt_in,
            "bias": b_in,
        }
        for c in range(N_CORES)
    ]

    kwargs = {}
    if TRACE:
        bass_utils.upload_artifacts = lambda tmpdir: tmpdir
        kwargs["trace"] = True
    res = bass_utils.run_bass_kernel_spmd(
        nc, in_maps, core_ids=list(range(N_CORES)), **kwargs
    )
    if TRACE:
        LAST_EXEC_NS = res.exec_time_ns
        if res.instructions_and_trace is not None:
            LAST_TRACE_PATH = res.instructions_and_trace[1]

    out = np.empty((N, OC, LEN), dtype=np.float32)
    for c in range(N_CORES):
        out[c] = res.results[c]["out"].reshape(OC, LEN)
    return out
